# revision 23
# baseline (speedup 1.0000x reference)
"""Trainium2 Bass kernel for nn_LinearAttention (RoPE(Q) @ RoPE(Q)^T @ V).

Algebra: no softmax, so out = (QR @ QR^T) @ V == QR @ (QR^T @ V) with a
[d,d] (64x64) intermediate per head. Sharding: 16 heads / 8 cores = 2
heads per core, no cross-core traffic. The two heads ride the two
64-lane halves of the 128x128 PE array.

Layout: t = p*16 + r*8 + c (p = SBUF partition, r = range 0/1, c =
chunk-in-range); the host packs/unpacks with this permutation.

v2 changes vs the first working kernel (31.1us):
  * Tables shipped COMPACT ([r,c,k] cos/sin, no per-head repeat: 288KB
    instead of 544KB) and broadcast over h with stride-0 APs in the
    RoPE muls -- cuts the input stream by 20%.
  * RoPE is DVE-only. GpSimd tensor ops are gone: every DVE
    tensor_tensor needs the shared SBUF read port that GpSimd locks
    for its whole (4x slower) instruction, so DVE+GpSimd elementwise
    work serializes instead of overlapping (measured 3x slowdown).
  * 12 RoPE ops of [128,512], all reads/writes contiguous (the chunk
    strides moved into the matmul lhsT APs, which tolerate them).
  * Transposes batch 4 chunks into one PSUM bank -> 4 big [128,512]
    evacuation casts on ACT instead of 16 small copies split DVE/ACT.
  * Phase-3 uses 4 distinct PSUM banks so its matmuls stream
    back-to-back; evac casts alternate DVE/ACT; output DMAs alternate
    the two HWDGE rings.
  * Input DMA instructions are hoisted into the engine preamble block
    (before the initial all-engine barrier) -- they have no waits, and
    issuing them ~0.8us earlier starts the HBM stream during the
    barrier/branch overhead.
  * PE warm-up spam sized to bridge from the preamble to the first
    real matmul, plus a short mid-kernel bridge while DVE finishes
    RoPE-B, so HAM stays at K=8/8 for the phase-3 matmuls.
"""

from contextlib import ExitStack

import numpy as np

import concourse.bass as bass
import concourse.mybir as mybir
import concourse.tile as tile
from concourse.bass_utils import run_bass_kernel_spmd
from concourse.vector_clock import ScopedClock

H, T, D = 16, 2048, 64
N_CORES = 8
HPC = H // N_CORES  # heads per core
P = 128
NT = T // P  # 16 t-chunks per head
HD = D // 2
NTAB = 2 * 2 * 8 * HD + P  # cosA|sinA|cosB|sinB ([c,k] each) | idt
F32 = mybir.dt.float32
BF16 = mybir.dt.bfloat16
N_WARM = 19  # leading dep-free matmuls: preamble -> first real MM
N_WARM_MID = 4  # bridge the PE gap while DVE finishes RoPE-B
SLIM_TAIL_NO_CLEAR = True  # skip the final sem clear + barrier


def _rope_tables():
    inv_freq = 1.0 / (10000.0 ** (np.arange(0, D, 2, dtype=np.float32) / D))
    t = np.arange(T, dtype=np.float32)
    freqs = np.outer(t, inv_freq).astype(np.float32)  # [T, D/2]
    return np.cos(freqs).astype(np.float32), np.sin(freqs).astype(np.float32)


class _SlimTileContext(tile.TileContext):
    """TileContext whose kernel tail uses per-engine drains + a
    sequencer-level (sem-only) barrier instead of the full EVSEM
    butterfly (~8us)."""

    def _drain_and_barrier(self, tick_clock, wait_clock):
        nc = self.nc
        drain_inst = nc.sync.drain()
        wait_clock.add_sem_waits(
            drain_inst.ins, ScopedClock({None: tick_clock.global_clock})
        )
        for eng in nc.engines.values():
            if eng.engine != mybir.EngineType.SP:
                eng.drain(fusable=False)
        nc.all_engine_barrier(sem_only=True)
        popped = nc._tile_sem_poison_stack.pop()
        assert popped is self._sem_poison
        if not SLIM_TAIL_NO_CLEAR:
            nc.clear_and_free_semaphores(list(self.sems.allocated().values()))
            nc.all_engine_barrier(sem_only=True)


def _build_nc():
    nc = bass.Bass()
    TAB = nc.declare_dram_parameter("TAB", [P, NTAB], BF16, isOutput=False)
    # q: [p, (x h c k)] per range (x = rotate-half half, h = head)
    QA = nc.declare_dram_parameter("QA", [P, 1024], BF16, isOutput=False)
    QB = nc.declare_dram_parameter("QB", [P, 1024], BF16, isOutput=False)
    # v: [p, (c h d)] per range
    VA = nc.declare_dram_parameter("VA", [P, 1024], BF16, isOutput=False)
    VB = nc.declare_dram_parameter("VB", [P, 1024], BF16, isOutput=False)
    OUT = nc.declare_dram_parameter("OUT", [P, T], BF16, isOutput=True)

    with _SlimTileContext(nc) as tc, ExitStack() as ctx:
        singles = ctx.enter_context(tc.tile_pool(name="singles", bufs=1))
        ps_s = ctx.enter_context(tc.tile_pool(name="ps_s", bufs=1, space="PSUM"))
        ps_tp = ctx.enter_context(tc.tile_pool(name="ps_tp", bufs=2, space="PSUM"))
        ps_o = ctx.enter_context(tc.tile_pool(name="ps_o", bufs=4, space="PSUM"))
        ps_w = ctx.enter_context(tc.tile_pool(name="ps_w", bufs=1, space="PSUM"))

        tab_sb = singles.tile([P, NTAB], BF16)
        # q layout: [p, r, x, h, c, k]
        q_sb = singles.tile([P, 2, 2, HPC, 8, HD], BF16)
        v_sb = singles.tile([P, NT, P], BF16)
        # chunk-major so each chunk's (h,x,k) is a contiguous 128-elem
        # lhsT slice (matmul stationary APs allow only one free dim)
        qr_sb = singles.tile([P, NT, HPC, 2, HD], BF16)
        tm = singles.tile([P, 4, HPC, 8, HD], BF16)
        qrt_sb = singles.tile([P, NT * P], BF16)
        s2d = singles.tile([P, P], BF16)
        outT_sb = singles.tile([P, T], BF16)
        spam_src = singles.tile([P, P], F32)

        # V streams on the third (SWDGE/GpSimd) DMA ring so Q owns the
        # sync ring and the tables own the scalar ring. Emitted first so
        # they sit at the head of the Pool queue, right after the
        # preamble barrier (they must NOT be hoisted before Pool's
        # barrier-gather, which would stall every engine).
        nc.gpsimd.dma_start(
            out=v_sb[:, 0:8], in_=VA[:].rearrange("p (c f) -> p c f", c=8)
        )
        nc.gpsimd.dma_start(
            out=v_sb[:, 8:16], in_=VB[:].rearrange("p (c f) -> p c f", c=8)
        )

        # spam seed on DVE (idle until RoPE); s2d off-diagonal zeros on
        # GpSimd after the V issues (needed only by phase 3, and GpSimd
        # ops must not overlap DVE tensor_tensor work -- shared port).
        nc.vector.memset(spam_src[:, 0:2], 0.0)
        nc.gpsimd.memset(s2d[0:D, D:P], 0.0)
        nc.gpsimd.memset(s2d[D:P, 0:D], 0.0)

        # Input DMAs, split fine so the earliest consumers unblock
        # sooner. sync: QA-lo, QA-hi, QB-lo, QB-hi, VB; scalar: TAB-A,
        # TAB-B+idt, VA. (_hoist_input_dmas moves all of these into the
        # preamble block so the HBM stream starts during the barrier.)
        def qview(dram, x):
            return dram[:, x * 512 : (x + 1) * 512].rearrange(
                "p (h c k) -> p h c k", h=HPC, c=8
            )

        nc.sync.dma_start(out=tab_sb[:, 0:512], in_=TAB[:, 0:512])
        nc.scalar.dma_start(out=tab_sb[:, 512:NTAB], in_=TAB[:, 512:NTAB])
        nc.sync.dma_start(out=q_sb[:, 0, 0], in_=qview(QA, 0))
        nc.scalar.dma_start(out=q_sb[:, 0, 1], in_=qview(QA, 1))
        nc.sync.dma_start(out=q_sb[:, 1, 0], in_=qview(QB, 0))
        nc.scalar.dma_start(out=q_sb[:, 1, 1], in_=qview(QB, 1))

        # Garbage-input PE warm-up: dep-free REGULAR matmuls (transpose
        # mode may not register as PE-busy for HAM) into rotating slices
        # of one preallocated PSUM bank -- slices avoid the tile-pool
        # recycling semaphores that would serialize the PE queue.
        spam_ps = ps_w.tile([P, 512], F32)
        for i in range(N_WARM):
            j = i % 4
            nc.tensor.matmul(
                spam_ps[:, j * P : (j + 1) * P], lhsT=spam_src, rhs=spam_src,
                start=True, stop=True, skip_group_check=True,
            )

        idt = tab_sb[:, 4 * 256 :]  # [P, 128] identity

        def rope(r):
            # 6 contiguous [128,512] DVE ops; cos/sin broadcast over h.
            cosB = (
                tab_sb[:, r * 512 : r * 512 + 256]
                .rearrange("p (c k) -> p c k", c=8)
                .unsqueeze(1)
                .to_broadcast([P, HPC, 8, HD])
            )
            sinB = (
                tab_sb[:, r * 512 + 256 : r * 512 + 512]
                .rearrange("p (c k) -> p c k", c=8)
                .unsqueeze(1)
                .to_broadcast([P, HPC, 8, HD])
            )
            qlo = q_sb[:, r, 0]
            qhi = q_sb[:, r, 1]
            cs = slice(r * 8, r * 8 + 8)
            # combine dests scatter into the chunk-major qr tile using
            # the same (h, c, k) iteration order as the contiguous srcs
            qr_lo = qr_sb[:, cs, :, 0, :].rearrange("p c h k -> p h c k")
            qr_hi = qr_sb[:, cs, :, 1, :].rearrange("p c h k -> p h c k")
            # q-lo muls first: the lo-half DMA lands ~0.7us before hi
            nc.vector.tensor_mul(tm[:, 0], qlo, cosB)
            nc.vector.tensor_mul(tm[:, 3], qlo, sinB)
            nc.vector.tensor_mul(tm[:, 1], qhi, sinB)
            nc.vector.tensor_sub(qr_lo, tm[:, 0], tm[:, 1])
            nc.vector.tensor_mul(tm[:, 2], qhi, cosB)
            nc.vector.tensor_add(qr_hi, tm[:, 2], tm[:, 3])

        s2_ps = ps_s.tile([P, P], F32)

        def phase2(r):
            # per chunk: one LDW (shared) + accum MM + transpose MM;
            # transposes batch 4 chunks per PSUM bank, evacuated by ACT
            # as one [128,512] cast each.
            for ci in range(8):
                c = r * 8 + ci
                if c % 4 == 0:
                    tp = ps_tp.tile([P, 512], F32, tag="tp")
                    phase2.tp = tp
                tp = phase2.tp
                qr_c = qr_sb[:, c].rearrange("p h x k -> p (h x k)")
                nc.tensor.matmul(
                    s2_ps, lhsT=qr_c, rhs=v_sb[:, c],
                    start=(c == 0), stop=(c == NT - 1),
                )
                j = c % 4
                nc.tensor.matmul(
                    tp[:, j * P : (j + 1) * P], lhsT=qr_c, rhs=idt,
                    start=True, stop=True,
                )
                if c % 4 == 3:
                    # ACT takes groups 0-2 (DVE busy with RoPE); DVE
                    # (free after RoPE-B) takes the last group
                    g = c // 4
                    dst = qrt_sb[:, g * 512 : (g + 1) * 512]
                    if g < 3:
                        nc.scalar.copy(out=dst, in_=tp)
                    else:
                        nc.vector.tensor_copy(out=dst, in_=tp)

        rope(0)
        phase2(0)
        rope(1)
        # Bridge the PE idle window while DVE finishes RoPE-B.
        for i in range(N_WARM_MID):
            j = i % 4
            nc.tensor.matmul(
                spam_ps[:, j * P : (j + 1) * P], lhsT=spam_src, rhs=spam_src,
                start=True, stop=True, skip_group_check=True,
            )
        phase2(1)

        # Diagonal S_h blocks -> block-diagonal phase-3 operand, one
        # cast per engine so they run in parallel.
        nc.scalar.copy(out=s2d[0:D, 0:D], in_=s2_ps[0:D, 0:D])
        nc.vector.tensor_copy(out=s2d[D:P, D:P], in_=s2_ps[D:P, D:P])

        # outT blocks: blockdiag(S)^T @ QRT serves both heads at once.
        # 4 distinct PSUM banks; evac casts alternate DVE/ACT; output
        # DMAs alternate the two rings.
        for i in range(4):
            o_ps = ps_o.tile([P, 512], F32, tag="o")
            blk = slice(i * 512, (i + 1) * 512)
            nc.tensor.matmul(
                o_ps, lhsT=s2d, rhs=qrt_sb[:, blk], start=True, stop=True
            )
            if i % 2 == 0:
                nc.vector.tensor_copy(out=outT_sb[:, blk], in_=o_ps)
                nc.sync.dma_start(out=OUT[:, blk], in_=outT_sb[:, blk])
            else:
                nc.scalar.copy(out=outT_sb[:, blk], in_=o_ps)
                nc.scalar.dma_start(out=OUT[:, blk], in_=outT_sb[:, blk])

    _split_multi_waits(nc)
    _hoist_input_dmas(nc)
    return nc


def _split_multi_waits(nc):
    """This compiler build rejects instructions carrying more than one
    sync-wait command: split extras into single-wait NoOps placed
    immediately before on the same engine."""
    n = 0
    for f in nc.m.functions:
        for blk in f.blocks:
            new_insts = []
            for inst in blk.instructions:
                si = inst.sync_info
                waits = list(si.on_wait) if si else []
                if len(waits) > 1:
                    for w in waits[:-1]:
                        nop = mybir.InstNoOp(name=f"W-split-{n}", ins=[], outs=[])
                        n += 1
                        nop.engine = inst.engine
                        nop.sync_info = mybir.SyncInfo(on_wait=[w], on_update=[])
                        new_insts.append(nop)
                    inst.sync_info = mybir.SyncInfo(
                        on_wait=[waits[-1]], on_update=list(si.on_update)
                    )
                new_insts.append(inst)
            blk.instructions = new_insts


def _hoist_input_dmas(nc):
    """Move the wait-free input DMA issues from the main block into the
    preamble block, right AFTER each issuing engine's barrier-increment
    Drain (a DMA issue costs ~0.65us on the issuing engine; placing it
    before the Drain would hold the all-engine barrier and stall every
    other engine behind the whole DMA issue train)."""
    f = nc.m.functions[0]
    if len(f.blocks) < 2:
        return
    pre, main = f.blocks[0], f.blocks[1]
    hoist = []
    for inst in list(main.instructions):
        if isinstance(inst, mybir.InstDMACopy):
            if inst.engine == mybir.EngineType.Pool:
                # Pool is the barrier gatherer: a DMA issue before its
                # gather EventSemaphores would stall every engine
                continue
            si = inst.sync_info
            if si is not None and len(si.on_wait) > 0:
                continue
            # input loads only: DRAM source (ins reference a DRAM tensor)
            srcs = [x.memref for x in inst.ins] if inst.ins else []
            if any(n.startswith(("QA", "QB", "VA", "VB", "TAB")) for n in srcs):
                hoist.append(inst)
    if not hoist:
        return
    for inst in hoist:
        main.instructions.remove(inst)
    # insert right after the issuing engine's first InstDrain (which
    # carries the barrier increment), preserving issue order per engine
    for inst in reversed(hoist):
        idx = next(
            (
                i
                for i, pi in enumerate(pre.instructions)
                if isinstance(pi, mybir.InstDrain) and pi.engine == inst.engine
            ),
            None,
        )
        if idx is None:
            main.instructions.insert(0, inst)
        else:
            pre.instructions.insert(idx + 1, inst)


_NC_CACHE = None


def _get_nc():
    global _NC_CACHE
    if _NC_CACHE is None:
        _NC_CACHE = _build_nc()
    return _NC_CACHE


def _pack_inputs(Qs, Vs, cos32, sin32, idt):
    import ml_dtypes

    bf16 = ml_dtypes.bfloat16

    # [T, X] -> [P, NT, X] with t = p*NT + u, u = r*8 + c
    def r(x):
        return x.reshape(P, NT, -1)

    # compact tables: tab[p, r, c, k] = cos((p*16 + r*8 + c) * w_k)
    ce = r(cos32).reshape(P, 2, 8, HD)  # [p, r, c, k]
    se = r(sin32).reshape(P, 2, 8, HD)
    tab = np.concatenate(
        [
            ce[:, 0].reshape(P, -1),
            se[:, 0].reshape(P, -1),
            ce[:, 1].reshape(P, -1),
            se[:, 1].reshape(P, -1),
            idt,
        ],
        axis=1,
    ).astype(bf16)
    tab = np.ascontiguousarray(tab)

    in_maps = []
    for core in range(N_CORES):
        h0 = core * HPC
        # q[p, r, x, h, c, k], v[p, c16, h, d]
        q = np.empty((P, 2, 2, HPC, 8, HD), np.float32)
        v = np.empty((P, NT, HPC, D), np.float32)
        for h in range(HPC):
            qh = r(Qs[h0 + h]).reshape(P, 2, 8, D)  # [p, r, c, d]
            q[:, :, 0, h] = qh[:, :, :, :HD]
            q[:, :, 1, h] = qh[:, :, :, HD:]
            v[:, :, h] = r(Vs[h0 + h])
        in_maps.append(
            {
                "TAB": tab,
                "QA": np.ascontiguousarray(q[:, 0].reshape(P, -1).astype(bf16)),
                "QB": np.ascontiguousarray(q[:, 1].reshape(P, -1).astype(bf16)),
                "VA": np.ascontiguousarray(
                    v[:, 0:8].reshape(P, -1).astype(bf16)
                ),
                "VB": np.ascontiguousarray(
                    v[:, 8:16].reshape(P, -1).astype(bf16)
                ),
            }
        )
    return in_maps


def _unpack_out(o):
    # o: [P, T] = outT; rows h*64+j, cols c-major: col = u*128 + f, t = f*16+u
    a = o.reshape(HPC, D, NT, P)  # [h, j, u, f]
    return a.transpose(0, 3, 2, 1).reshape(HPC, T, D)  # [h, t=f*16+u, j]


def run_inner(Q, K, V, trace=False):
    del K  # the module sets KR = QR; K is unused
    Qs = np.asarray(Q, dtype=np.float32)[0]  # [H, T, D]
    Vs = np.asarray(V, dtype=np.float32)[0]
    cos32, sin32 = _rope_tables()
    idt = np.eye(P, dtype=np.float32)
    nc = _get_nc()
    in_maps = _pack_inputs(Qs, Vs, cos32, sin32, idt)
    res = run_bass_kernel_spmd(nc, in_maps, list(range(N_CORES)), trace=trace)
    outs = [_unpack_out(np.asarray(res.results[i]["OUT"])) for i in range(N_CORES)]
    out = np.concatenate(outs, axis=0)[None]  # [1, H, T, D]
    return out.astype(np.float32), res


def kernel(Q, K, V):
    out, _ = run_inner(Q, K, V, trace=False)
    return out


# revision 25
# speedup vs baseline: 1.1744x; 1.1744x over previous
"""Trainium2 Bass kernel for nn_LinearAttention (RoPE(Q) @ RoPE(Q)^T @ V).

Algebra: no softmax, so out = (QR @ QR^T) @ V == QR @ (QR^T @ V) with a
[d,d] (64x64) intermediate per head. Sharding: 16 heads / 8 cores = 2
heads per core, no cross-core traffic. The two heads ride the two
64-lane halves of the 128x128 PE array.

Layout: t = p*16 + r*8 + c (p = SBUF partition, r = range 0/1, c =
chunk-in-range); the host packs/unpacks with this permutation.

v2 changes vs the first working kernel (31.1us):
  * Tables shipped COMPACT ([r,c,k] cos/sin, no per-head repeat: 288KB
    instead of 544KB) and broadcast over h with stride-0 APs in the
    RoPE muls -- cuts the input stream by 20%.
  * RoPE is DVE-only. GpSimd tensor ops are gone: every DVE
    tensor_tensor needs the shared SBUF read port that GpSimd locks
    for its whole (4x slower) instruction, so DVE+GpSimd elementwise
    work serializes instead of overlapping (measured 3x slowdown).
  * 12 RoPE ops of [128,512], all reads/writes contiguous (the chunk
    strides moved into the matmul lhsT APs, which tolerate them).
  * Transposes batch 4 chunks into one PSUM bank -> 4 big [128,512]
    evacuation casts on ACT instead of 16 small copies split DVE/ACT.
  * Phase-3 uses 4 distinct PSUM banks so its matmuls stream
    back-to-back; evac casts alternate DVE/ACT; output DMAs alternate
    the two HWDGE rings.
  * Input DMA instructions are hoisted into the engine preamble block
    (before the initial all-engine barrier) -- they have no waits, and
    issuing them ~0.8us earlier starts the HBM stream during the
    barrier/branch overhead.
  * PE warm-up spam sized to bridge from the preamble to the first
    real matmul, plus a short mid-kernel bridge while DVE finishes
    RoPE-B, so HAM stays at K=8/8 for the phase-3 matmuls.
"""

from contextlib import ExitStack

import numpy as np

import concourse.bass as bass
import concourse.mybir as mybir
import concourse.tile as tile
from concourse.bass_utils import run_bass_kernel_spmd
from concourse.vector_clock import ScopedClock

H, T, D = 16, 2048, 64
N_CORES = 8
HPC = H // N_CORES  # heads per core
P = 128
NT = T // P  # 16 t-chunks per head
HD = D // 2
NTAB = 2 * 2 * 8 * HD + P  # cosA|sinA|cosB|sinB ([c,k] each) | idt
F32 = mybir.dt.float32
BF16 = mybir.dt.bfloat16
N_WARM = 19  # leading dep-free matmuls: preamble -> first real MM
N_WARM_MID = 4  # bridge the PE gap while DVE finishes RoPE-B
SLIM_TAIL_NO_CLEAR = True  # skip the final sem clear + barrier


def _rope_tables():
    inv_freq = 1.0 / (10000.0 ** (np.arange(0, D, 2, dtype=np.float32) / D))
    t = np.arange(T, dtype=np.float32)
    freqs = np.outer(t, inv_freq).astype(np.float32)  # [T, D/2]
    return np.cos(freqs).astype(np.float32), np.sin(freqs).astype(np.float32)


class _SlimTileContext(tile.TileContext):
    """TileContext whose kernel tail uses per-engine drains + a
    sequencer-level (sem-only) barrier instead of the full EVSEM
    butterfly (~8us)."""

    def _drain_and_barrier(self, tick_clock, wait_clock):
        nc = self.nc
        drain_inst = nc.sync.drain()
        wait_clock.add_sem_waits(
            drain_inst.ins, ScopedClock({None: tick_clock.global_clock})
        )
        for eng in nc.engines.values():
            if eng.engine != mybir.EngineType.SP:
                eng.drain(fusable=False)
        nc.all_engine_barrier(sem_only=True)
        popped = nc._tile_sem_poison_stack.pop()
        assert popped is self._sem_poison
        if not SLIM_TAIL_NO_CLEAR:
            nc.clear_and_free_semaphores(list(self.sems.allocated().values()))
            nc.all_engine_barrier(sem_only=True)


def _build_nc():
    nc = bass.Bass()
    TAB = nc.declare_dram_parameter("TAB", [P, NTAB], BF16, isOutput=False)
    # q: [p, (x h c k)] per range (x = rotate-half half, h = head)
    QA = nc.declare_dram_parameter("QA", [P, 1024], BF16, isOutput=False)
    QB = nc.declare_dram_parameter("QB", [P, 1024], BF16, isOutput=False)
    # v: [p, (c h d)] per range
    VA = nc.declare_dram_parameter("VA", [P, 1024], BF16, isOutput=False)
    VB = nc.declare_dram_parameter("VB", [P, 1024], BF16, isOutput=False)
    OUT = nc.declare_dram_parameter("OUT", [P, T], BF16, isOutput=True)

    with _SlimTileContext(nc) as tc, ExitStack() as ctx:
        singles = ctx.enter_context(tc.tile_pool(name="singles", bufs=1))
        ps_s = ctx.enter_context(tc.tile_pool(name="ps_s", bufs=1, space="PSUM"))
        ps_tp = ctx.enter_context(tc.tile_pool(name="ps_tp", bufs=2, space="PSUM"))
        ps_o = ctx.enter_context(tc.tile_pool(name="ps_o", bufs=4, space="PSUM"))
        ps_w = ctx.enter_context(tc.tile_pool(name="ps_w", bufs=1, space="PSUM"))

        tab_sb = singles.tile([P, NTAB], BF16)
        # q layout: [p, r, x, h, c, k]
        q_sb = singles.tile([P, 2, 2, HPC, 8, HD], BF16)
        v_sb = singles.tile([P, NT, P], BF16)
        # chunk-major so each chunk's (h,x,k) is a contiguous 128-elem
        # lhsT slice (matmul stationary APs allow only one free dim)
        qr_sb = singles.tile([P, NT, HPC, 2, HD], BF16)
        tm = singles.tile([P, 4, HPC, 8, HD], BF16)
        qrt_sb = singles.tile([P, NT * P], BF16)
        s2d = singles.tile([P, P], BF16)
        outT_sb = singles.tile([P, T], BF16)
        spam_src = singles.tile([P, P], F32)

        # V streams on the third (SWDGE/GpSimd) DMA ring so Q owns the
        # sync ring and the tables own the scalar ring. Emitted first so
        # they sit at the head of the Pool queue, right after the
        # preamble barrier (they must NOT be hoisted before Pool's
        # barrier-gather, which would stall every engine).
        nc.gpsimd.dma_start(
            out=v_sb[:, 8:16], in_=VB[:].rearrange("p (c f) -> p c f", c=8)
        )

        # spam seed on DVE (idle until RoPE); s2d off-diagonal zeros on
        # GpSimd after the V issues (needed only by phase 3, and GpSimd
        # ops must not overlap DVE tensor_tensor work -- shared port).
        nc.vector.memset(spam_src[:, 0:2], 0.0)
        nc.gpsimd.memset(s2d[0:D, D:P], 0.0)
        nc.gpsimd.memset(s2d[D:P, 0:D], 0.0)

        # Input DMAs, split fine so the earliest consumers unblock
        # sooner. sync: QA-lo, QA-hi, QB-lo, QB-hi, VB; scalar: TAB-A,
        # TAB-B+idt, VA. (_hoist_input_dmas moves all of these into the
        # preamble block so the HBM stream starts during the barrier.)
        def qview(dram, x):
            return dram[:, x * 512 : (x + 1) * 512].rearrange(
                "p (h c k) -> p h c k", h=HPC, c=8
            )

        nc.sync.dma_start(out=q_sb[:, 0, 0], in_=qview(QA, 0))
        nc.scalar.dma_start(out=tab_sb[:, 0:512], in_=TAB[:, 0:512])
        nc.sync.dma_start(out=q_sb[:, 0, 1], in_=qview(QA, 1))
        nc.scalar.dma_start(out=tab_sb[:, 512:NTAB], in_=TAB[:, 512:NTAB])
        nc.sync.dma_start(out=q_sb[:, 1, 0], in_=qview(QB, 0))
        nc.scalar.dma_start(
            out=v_sb[:, 0:8], in_=VA[:].rearrange("p (c f) -> p c f", c=8)
        )
        nc.sync.dma_start(out=q_sb[:, 1, 1], in_=qview(QB, 1))

        # Garbage-input PE warm-up: dep-free REGULAR matmuls (transpose
        # mode may not register as PE-busy for HAM) into rotating slices
        # of one preallocated PSUM bank -- slices avoid the tile-pool
        # recycling semaphores that would serialize the PE queue.
        spam_ps = ps_w.tile([P, 512], F32)
        for i in range(N_WARM):
            j = i % 4
            nc.tensor.matmul(
                spam_ps[:, j * P : (j + 1) * P], lhsT=spam_src, rhs=spam_src,
                start=True, stop=True, skip_group_check=True,
            )

        idt = tab_sb[:, 4 * 256 :]  # [P, 128] identity

        def rope(r):
            # 6 contiguous [128,512] DVE ops; cos/sin broadcast over h.
            cosB = (
                tab_sb[:, r * 512 : r * 512 + 256]
                .rearrange("p (c k) -> p c k", c=8)
                .unsqueeze(1)
                .to_broadcast([P, HPC, 8, HD])
            )
            sinB = (
                tab_sb[:, r * 512 + 256 : r * 512 + 512]
                .rearrange("p (c k) -> p c k", c=8)
                .unsqueeze(1)
                .to_broadcast([P, HPC, 8, HD])
            )
            qlo = q_sb[:, r, 0]
            qhi = q_sb[:, r, 1]
            cs = slice(r * 8, r * 8 + 8)
            # combine dests scatter into the chunk-major qr tile using
            # the same (h, c, k) iteration order as the contiguous srcs
            qr_lo = qr_sb[:, cs, :, 0, :].rearrange("p c h k -> p h c k")
            qr_hi = qr_sb[:, cs, :, 1, :].rearrange("p c h k -> p h c k")
            # q-lo muls first: the lo-half DMA lands ~0.7us before hi
            nc.vector.tensor_mul(tm[:, 0], qlo, cosB)
            nc.vector.tensor_mul(tm[:, 3], qlo, sinB)
            nc.vector.tensor_mul(tm[:, 1], qhi, sinB)
            nc.vector.tensor_sub(qr_lo, tm[:, 0], tm[:, 1])
            nc.vector.tensor_mul(tm[:, 2], qhi, cosB)
            nc.vector.tensor_add(qr_hi, tm[:, 2], tm[:, 3])

        s2_ps = ps_s.tile([P, P], F32)

        def phase2(r):
            # per chunk: one LDW (shared) + accum MM + transpose MM;
            # transposes batch 4 chunks per PSUM bank, evacuated by ACT
            # as one [128,512] cast each.
            for ci in range(8):
                c = r * 8 + ci
                if c % 4 == 0:
                    tp = ps_tp.tile([P, 512], F32, tag="tp")
                    phase2.tp = tp
                tp = phase2.tp
                qr_c = qr_sb[:, c].rearrange("p h x k -> p (h x k)")
                nc.tensor.matmul(
                    s2_ps, lhsT=qr_c, rhs=v_sb[:, c],
                    start=(c == 0), stop=(c == NT - 1),
                )
                j = c % 4
                nc.tensor.matmul(
                    tp[:, j * P : (j + 1) * P], lhsT=qr_c, rhs=idt,
                    start=True, stop=True,
                )
                if c % 4 == 3:
                    # ACT takes groups 0-2 (DVE busy with RoPE); DVE
                    # (free after RoPE-B) takes the last group
                    g = c // 4
                    dst = qrt_sb[:, g * 512 : (g + 1) * 512]
                    if g < 3:
                        nc.scalar.copy(out=dst, in_=tp)
                    else:
                        nc.vector.tensor_copy(out=dst, in_=tp)

        rope(0)
        phase2(0)
        rope(1)
        # Bridge the PE idle window while DVE finishes RoPE-B.
        for i in range(N_WARM_MID):
            j = i % 4
            nc.tensor.matmul(
                spam_ps[:, j * P : (j + 1) * P], lhsT=spam_src, rhs=spam_src,
                start=True, stop=True, skip_group_check=True,
            )
        phase2(1)

        # Diagonal S_h blocks -> block-diagonal phase-3 operand, one
        # cast per engine so they run in parallel.
        nc.scalar.copy(out=s2d[0:D, 0:D], in_=s2_ps[0:D, 0:D])
        nc.vector.tensor_copy(out=s2d[D:P, D:P], in_=s2_ps[D:P, D:P])

        # outT blocks: blockdiag(S)^T @ QRT serves both heads at once.
        # 4 distinct PSUM banks; evac casts alternate DVE/ACT; output
        # DMAs alternate the two rings.
        for i in range(4):
            o_ps = ps_o.tile([P, 512], F32, tag="o")
            blk = slice(i * 512, (i + 1) * 512)
            nc.tensor.matmul(
                o_ps, lhsT=s2d, rhs=qrt_sb[:, blk], start=True, stop=True
            )
            if i % 2 == 0:
                nc.vector.tensor_copy(out=outT_sb[:, blk], in_=o_ps)
                nc.sync.dma_start(out=OUT[:, blk], in_=outT_sb[:, blk])
            else:
                nc.scalar.copy(out=outT_sb[:, blk], in_=o_ps)
                nc.scalar.dma_start(out=OUT[:, blk], in_=outT_sb[:, blk])

    _split_multi_waits(nc)
    _hoist_input_dmas(nc)
    return nc


def _split_multi_waits(nc):
    """This compiler build rejects instructions carrying more than one
    sync-wait command: split extras into single-wait NoOps placed
    immediately before on the same engine."""
    n = 0
    for f in nc.m.functions:
        for blk in f.blocks:
            new_insts = []
            for inst in blk.instructions:
                si = inst.sync_info
                waits = list(si.on_wait) if si else []
                if len(waits) > 1:
                    for w in waits[:-1]:
                        nop = mybir.InstNoOp(name=f"W-split-{n}", ins=[], outs=[])
                        n += 1
                        nop.engine = inst.engine
                        nop.sync_info = mybir.SyncInfo(on_wait=[w], on_update=[])
                        new_insts.append(nop)
                    inst.sync_info = mybir.SyncInfo(
                        on_wait=[waits[-1]], on_update=list(si.on_update)
                    )
                new_insts.append(inst)
            blk.instructions = new_insts


def _hoist_input_dmas(nc):
    """Move the wait-free input DMA issues from the main block into the
    preamble block, right AFTER each issuing engine's barrier-increment
    Drain (a DMA issue costs ~0.65us on the issuing engine; placing it
    before the Drain would hold the all-engine barrier and stall every
    other engine behind the whole DMA issue train)."""
    f = nc.m.functions[0]
    if len(f.blocks) < 2:
        return
    pre, main = f.blocks[0], f.blocks[1]
    hoist = []
    for inst in list(main.instructions):
        if isinstance(inst, mybir.InstDMACopy):
            if inst.engine == mybir.EngineType.Pool:
                # Pool is the barrier gatherer: a DMA issue before its
                # gather EventSemaphores would stall every engine
                continue
            si = inst.sync_info
            if si is not None and len(si.on_wait) > 0:
                continue
            # input loads only: DRAM source (ins reference a DRAM tensor)
            srcs = [x.memref for x in inst.ins] if inst.ins else []
            if any(n.startswith(("QA", "QB", "VA", "VB", "TAB")) for n in srcs):
                hoist.append(inst)
    if not hoist:
        return
    for inst in hoist:
        main.instructions.remove(inst)
    # insert right after the issuing engine's first InstDrain (which
    # carries the barrier increment), preserving issue order per engine
    for inst in reversed(hoist):
        idx = next(
            (
                i
                for i, pi in enumerate(pre.instructions)
                if isinstance(pi, mybir.InstDrain) and pi.engine == inst.engine
            ),
            None,
        )
        if idx is None:
            main.instructions.insert(0, inst)
        else:
            pre.instructions.insert(idx + 1, inst)


_NC_CACHE = None


def _get_nc():
    global _NC_CACHE
    if _NC_CACHE is None:
        _NC_CACHE = _build_nc()
    return _NC_CACHE


def _pack_inputs(Qs, Vs, cos32, sin32, idt):
    import ml_dtypes

    bf16 = ml_dtypes.bfloat16

    # [T, X] -> [P, NT, X] with t = p*NT + u, u = r*8 + c
    def r(x):
        return x.reshape(P, NT, -1)

    # compact tables: tab[p, r, c, k] = cos((p*16 + r*8 + c) * w_k)
    ce = r(cos32).reshape(P, 2, 8, HD)  # [p, r, c, k]
    se = r(sin32).reshape(P, 2, 8, HD)
    tab = np.concatenate(
        [
            ce[:, 0].reshape(P, -1),
            se[:, 0].reshape(P, -1),
            ce[:, 1].reshape(P, -1),
            se[:, 1].reshape(P, -1),
            idt,
        ],
        axis=1,
    ).astype(bf16)
    tab = np.ascontiguousarray(tab)

    in_maps = []
    for core in range(N_CORES):
        h0 = core * HPC
        # q[p, r, x, h, c, k], v[p, c16, h, d]
        q = np.empty((P, 2, 2, HPC, 8, HD), np.float32)
        v = np.empty((P, NT, HPC, D), np.float32)
        for h in range(HPC):
            qh = r(Qs[h0 + h]).reshape(P, 2, 8, D)  # [p, r, c, d]
            q[:, :, 0, h] = qh[:, :, :, :HD]
            q[:, :, 1, h] = qh[:, :, :, HD:]
            v[:, :, h] = r(Vs[h0 + h])
        in_maps.append(
            {
                "TAB": tab,
                "QA": np.ascontiguousarray(q[:, 0].reshape(P, -1).astype(bf16)),
                "QB": np.ascontiguousarray(q[:, 1].reshape(P, -1).astype(bf16)),
                "VA": np.ascontiguousarray(
                    v[:, 0:8].reshape(P, -1).astype(bf16)
                ),
                "VB": np.ascontiguousarray(
                    v[:, 8:16].reshape(P, -1).astype(bf16)
                ),
            }
        )
    return in_maps


def _unpack_out(o):
    # o: [P, T] = outT; rows h*64+j, cols c-major: col = u*128 + f, t = f*16+u
    a = o.reshape(HPC, D, NT, P)  # [h, j, u, f]
    return a.transpose(0, 3, 2, 1).reshape(HPC, T, D)  # [h, t=f*16+u, j]


def run_inner(Q, K, V, trace=False):
    del K  # the module sets KR = QR; K is unused
    Qs = np.asarray(Q, dtype=np.float32)[0]  # [H, T, D]
    Vs = np.asarray(V, dtype=np.float32)[0]
    cos32, sin32 = _rope_tables()
    idt = np.eye(P, dtype=np.float32)
    nc = _get_nc()
    in_maps = _pack_inputs(Qs, Vs, cos32, sin32, idt)
    res = run_bass_kernel_spmd(nc, in_maps, list(range(N_CORES)), trace=trace)
    outs = [_unpack_out(np.asarray(res.results[i]["OUT"])) for i in range(N_CORES)]
    out = np.concatenate(outs, axis=0)[None]  # [1, H, T, D]
    return out.astype(np.float32), res


def kernel(Q, K, V):
    out, _ = run_inner(Q, K, V, trace=False)
    return out


# revision 26
# speedup vs baseline: 1.2165x; 1.0358x over previous
"""Trainium2 Bass kernel for nn_LinearAttention (RoPE(Q) @ RoPE(Q)^T @ V).

Algebra: no softmax, so out = (QR @ QR^T) @ V == QR @ (QR^T @ V) with a
[d,d] (64x64) intermediate per head. Sharding: 16 heads / 8 cores = 2
heads per core, no cross-core traffic; the two heads ride the two
64-lane halves of the 128x128 PE array. t = p*16 + r*8 + c (p = SBUF
partition, r = range, c = chunk); host packs/unpacks this permutation.

Key measured facts this version is built around (see trace notes):
  * Per DMA ring, each transfer costs ~1.8-2.8us end-to-end regardless
    of size (completion receipt does not pipeline), so inputs are
    grouped into at most 2 transfers per ring across 3 rings (sync,
    scalar-HWDGE, gpsimd-SWDGE), ordered by need:
      sync:   [cosA|sinA|qA-lo]  then [qB-lo]   (+ out blocks 0,2)
      scalar: [cosB|sinB|idt|qA-hi] then [qB-hi] (+ out blocks 1,3)
      gpsimd: [vA|vB]
  * Input DMA issues (~0.65us each on the issuing engine) are hoisted
    into the preamble block AFTER each engine's barrier-increment Drain
    (before it they stall every engine behind the issue train).
  * DVE tensor_tensor always needs the shared SBUF port that GpSimd
    locks per-instruction, so RoPE is DVE-only; cos/sin ship compact
    ([c,k], no per-head repeat) and broadcast over h via stride-0 APs.
  * matmul stationary APs allow one free dim -> qr is chunk-major,
    RoPE combines write strided.
  * Tile-pool recycling inserts completion semaphores: warm-up spam
    writes rotating slices of ONE preallocated PSUM bank instead.
  * PSUM evacuations batch 4 transpose chunks per bank and balance
    ACT/DVE; phase-3 uses 4 distinct PSUM banks.
  * Kernel tail: per-engine drains + one sem-only barrier, no sem
    clears (verified safe across re-executions).
"""

from contextlib import ExitStack

import numpy as np

import concourse.bass as bass
import concourse.mybir as mybir
import concourse.tile as tile
from concourse.bass_utils import run_bass_kernel_spmd
from concourse.vector_clock import ScopedClock

H, T, D = 16, 2048, 64
N_CORES = 8
HPC = H // N_CORES  # heads per core
P = 128
NT = T // P  # 16 t-chunks per core-head
HD = D // 2
F32 = mybir.dt.float32
BF16 = mybir.dt.bfloat16
N_WARM = 19  # leading dep-free matmuls: preamble -> first real MM
N_WARM_MID = 4  # bridge the PE gap while DVE finishes RoPE-B

# in_sb layout (elements per partition)
O_COSA, O_QALO = 0, 512
O_COSB, O_IDT, O_QAHI = 1024, 1536, 1664
O_QBLO, O_QBHI, O_V = 2176, 2688, 3200
N_IN = O_V + 2048


def _rope_tables():
    inv_freq = 1.0 / (10000.0 ** (np.arange(0, D, 2, dtype=np.float32) / D))
    t = np.arange(T, dtype=np.float32)
    freqs = np.outer(t, inv_freq).astype(np.float32)  # [T, D/2]
    return np.cos(freqs).astype(np.float32), np.sin(freqs).astype(np.float32)


class _SlimTileContext(tile.TileContext):
    """TileContext whose kernel tail uses per-engine drains + a single
    sem-only barrier instead of the full EVSEM butterfly (~8us) and the
    sem-clear chain (re-execution verified identical without it)."""

    def _drain_and_barrier(self, tick_clock, wait_clock):
        nc = self.nc
        drain_inst = nc.sync.drain()
        wait_clock.add_sem_waits(
            drain_inst.ins, ScopedClock({None: tick_clock.global_clock})
        )
        for eng in nc.engines.values():
            if eng.engine != mybir.EngineType.SP:
                eng.drain(fusable=False)
        nc.all_engine_barrier(sem_only=True)
        popped = nc._tile_sem_poison_stack.pop()
        assert popped is self._sem_poison


def _build_nc():
    nc = bass.Bass()
    G1 = nc.declare_dram_parameter("G1", [P, 1024], BF16, isOutput=False)
    G2 = nc.declare_dram_parameter("G2", [P, 1152], BF16, isOutput=False)
    G3 = nc.declare_dram_parameter("G3", [P, 512], BF16, isOutput=False)
    G4 = nc.declare_dram_parameter("G4", [P, 512], BF16, isOutput=False)
    VV = nc.declare_dram_parameter("VV", [P, 2048], BF16, isOutput=False)
    OUT = nc.declare_dram_parameter("OUT", [P, T], BF16, isOutput=True)

    with _SlimTileContext(nc) as tc, ExitStack() as ctx:
        singles = ctx.enter_context(tc.tile_pool(name="singles", bufs=1))
        ps_s = ctx.enter_context(tc.tile_pool(name="ps_s", bufs=1, space="PSUM"))
        ps_tp = ctx.enter_context(tc.tile_pool(name="ps_tp", bufs=2, space="PSUM"))
        ps_o = ctx.enter_context(tc.tile_pool(name="ps_o", bufs=4, space="PSUM"))
        ps_w = ctx.enter_context(tc.tile_pool(name="ps_w", bufs=1, space="PSUM"))

        in_sb = singles.tile([P, N_IN], BF16)
        # chunk-major so each chunk's (h,x,k) is a contiguous 128-elem
        # lhsT slice (matmul stationary APs allow only one free dim)
        qr_sb = singles.tile([P, NT, HPC, 2, HD], BF16)
        tm = singles.tile([P, 4, HPC, 8, HD], BF16)
        qrt_sb = singles.tile([P, NT * P], BF16)
        s2d = singles.tile([P, P], BF16)
        outT_sb = singles.tile([P, T], BF16)
        spam_src = singles.tile([P, P], F32)

        # V on the third (SWDGE) ring, at the head of the Pool queue --
        # it must NOT be hoisted before Pool's barrier-gather.
        nc.gpsimd.dma_start(
            out=in_sb[:, O_V : O_V + 2048], in_=VV[:]
        )

        # spam seed on DVE (idle until RoPE); s2d off-diagonal zeros on
        # GpSimd after the V issue (GpSimd must not overlap DVE
        # tensor_tensor work -- shared SBUF port).
        nc.vector.memset(spam_src[:, 0:2], 0.0)
        nc.gpsimd.memset(s2d[0:D, D:P], 0.0)
        nc.gpsimd.memset(s2d[D:P, 0:D], 0.0)

        # Two grouped transfers per HWDGE ring, hoisted into the
        # preamble block by _hoist_input_dmas.
        nc.sync.dma_start(out=in_sb[:, 0:1024], in_=G1[:])
        nc.scalar.dma_start(out=in_sb[:, 1024:2176], in_=G2[:])
        nc.sync.dma_start(out=in_sb[:, O_QBLO : O_QBLO + 512], in_=G3[:])
        nc.scalar.dma_start(out=in_sb[:, O_QBHI : O_QBHI + 512], in_=G4[:])

        # Garbage-input PE warm-up: dep-free REGULAR matmuls into
        # rotating slices of one preallocated PSUM bank (slices avoid
        # tile-pool recycling semaphores, which would serialize the PE
        # queue at ~531ns/op).
        spam_ps = ps_w.tile([P, 512], F32)
        for i in range(N_WARM):
            j = i % 4
            nc.tensor.matmul(
                spam_ps[:, j * P : (j + 1) * P], lhsT=spam_src, rhs=spam_src,
                start=True, stop=True, skip_group_check=True,
            )

        idt = in_sb[:, O_IDT : O_IDT + 128]
        v_view = in_sb[:, O_V : O_V + 2048].rearrange("p (c f) -> p c f", c=NT)

        def rope(r):
            # 6 contiguous [128,512] DVE ops; cos/sin broadcast over h.
            ocos = O_COSA if r == 0 else O_COSB
            cosB = (
                in_sb[:, ocos : ocos + 256]
                .rearrange("p (c k) -> p c k", c=8)
                .unsqueeze(1)
                .to_broadcast([P, HPC, 8, HD])
            )
            sinB = (
                in_sb[:, ocos + 256 : ocos + 512]
                .rearrange("p (c k) -> p c k", c=8)
                .unsqueeze(1)
                .to_broadcast([P, HPC, 8, HD])
            )
            olo = O_QALO if r == 0 else O_QBLO
            ohi = O_QAHI if r == 0 else O_QBHI
            qlo = in_sb[:, olo : olo + 512].rearrange(
                "p (h c k) -> p h c k", h=HPC, c=8
            )
            qhi = in_sb[:, ohi : ohi + 512].rearrange(
                "p (h c k) -> p h c k", h=HPC, c=8
            )
            cs = slice(r * 8, r * 8 + 8)
            # combine dests scatter into the chunk-major qr tile in the
            # same (h, c, k) iteration order as the contiguous srcs
            qr_lo = qr_sb[:, cs, :, 0, :].rearrange("p c h k -> p h c k")
            qr_hi = qr_sb[:, cs, :, 1, :].rearrange("p c h k -> p h c k")
            # q-lo muls first: the lo-half transfer lands first
            nc.vector.tensor_mul(tm[:, 0], qlo, cosB)
            nc.vector.tensor_mul(tm[:, 3], qlo, sinB)
            nc.vector.tensor_mul(tm[:, 1], qhi, sinB)
            nc.vector.tensor_sub(qr_lo, tm[:, 0], tm[:, 1])
            nc.vector.tensor_mul(tm[:, 2], qhi, cosB)
            nc.vector.tensor_add(qr_hi, tm[:, 2], tm[:, 3])

        s2_ps = ps_s.tile([P, P], F32)

        def phase2(r):
            # per chunk: one (shared) LDW + transpose MM + accum MM;
            # transposes batch 4 chunks per PSUM bank
            for ci in range(8):
                c = r * 8 + ci
                if c % 4 == 0:
                    phase2.tp = ps_tp.tile([P, 512], F32, tag="tp")
                tp = phase2.tp
                qr_c = qr_sb[:, c].rearrange("p h x k -> p (h x k)")
                j = c % 4
                # transpose first: a late V then doesn't stall the
                # transpose/evac pipeline behind it in the PE FIFO
                nc.tensor.matmul(
                    tp[:, j * P : (j + 1) * P], lhsT=qr_c, rhs=idt,
                    start=True, stop=True,
                )
                nc.tensor.matmul(
                    s2_ps, lhsT=qr_c, rhs=v_view[:, c],
                    start=(c == 0), stop=(c == NT - 1),
                )
                if c % 4 == 3:
                    # ACT takes groups 0-2 (DVE busy with RoPE); DVE
                    # (free after RoPE-B) takes the last group
                    g = c // 4
                    dst = qrt_sb[:, g * 512 : (g + 1) * 512]
                    if g < 3:
                        nc.scalar.copy(out=dst, in_=tp)
                    else:
                        nc.vector.tensor_copy(out=dst, in_=tp)

        rope(0)
        phase2(0)
        rope(1)
        # Bridge the PE idle window while DVE finishes RoPE-B.
        for i in range(N_WARM_MID):
            j = i % 4
            nc.tensor.matmul(
                spam_ps[:, j * P : (j + 1) * P], lhsT=spam_src, rhs=spam_src,
                start=True, stop=True, skip_group_check=True,
            )
        phase2(1)

        # Diagonal S_h blocks -> block-diagonal phase-3 operand, one
        # cast per engine so they run in parallel.
        nc.scalar.copy(out=s2d[0:D, 0:D], in_=s2_ps[0:D, 0:D])
        nc.vector.tensor_copy(out=s2d[D:P, D:P], in_=s2_ps[D:P, D:P])

        # outT blocks: blockdiag(S)^T @ QRT serves both heads at once.
        # 4 distinct PSUM banks; evac casts alternate DVE/ACT; output
        # DMAs alternate the two HWDGE rings.
        for i in range(4):
            o_ps = ps_o.tile([P, 512], F32, tag="o")
            blk = slice(i * 512, (i + 1) * 512)
            nc.tensor.matmul(
                o_ps, lhsT=s2d, rhs=qrt_sb[:, blk], start=True, stop=True
            )
            if i % 2 == 0:
                nc.vector.tensor_copy(out=outT_sb[:, blk], in_=o_ps)
                nc.sync.dma_start(out=OUT[:, blk], in_=outT_sb[:, blk])
            else:
                nc.scalar.copy(out=outT_sb[:, blk], in_=o_ps)
                nc.scalar.dma_start(out=OUT[:, blk], in_=outT_sb[:, blk])

    _split_multi_waits(nc)
    _hoist_input_dmas(nc)
    return nc


def _split_multi_waits(nc):
    """This compiler build rejects instructions carrying more than one
    sync-wait command: split extras into single-wait NoOps placed
    immediately before on the same engine."""
    n = 0
    for f in nc.m.functions:
        for blk in f.blocks:
            new_insts = []
            for inst in blk.instructions:
                si = inst.sync_info
                waits = list(si.on_wait) if si else []
                if len(waits) > 1:
                    for w in waits[:-1]:
                        nop = mybir.InstNoOp(name=f"W-split-{n}", ins=[], outs=[])
                        n += 1
                        nop.engine = inst.engine
                        nop.sync_info = mybir.SyncInfo(on_wait=[w], on_update=[])
                        new_insts.append(nop)
                    inst.sync_info = mybir.SyncInfo(
                        on_wait=[waits[-1]], on_update=list(si.on_update)
                    )
                new_insts.append(inst)
            blk.instructions = new_insts


def _hoist_input_dmas(nc):
    """Move the wait-free input DMA issues from the main block into the
    preamble block, right AFTER each issuing engine's barrier-increment
    Drain (a DMA issue costs ~0.65us on the issuing engine; placing it
    before the Drain would hold the all-engine barrier and stall every
    other engine behind the whole DMA issue train)."""
    f = nc.m.functions[0]
    if len(f.blocks) < 2:
        return
    pre, main = f.blocks[0], f.blocks[1]
    hoist = []
    for inst in list(main.instructions):
        if isinstance(inst, mybir.InstDMACopy):
            if inst.engine == mybir.EngineType.Pool:
                # Pool is the barrier gatherer: a DMA issue before its
                # gather EventSemaphores would stall every engine
                continue
            si = inst.sync_info
            if si is not None and len(si.on_wait) > 0:
                continue
            srcs = [x.memref for x in inst.ins] if inst.ins else []
            if any(n.startswith(("G1", "G2", "G3", "G4")) for n in srcs):
                hoist.append(inst)
    if not hoist:
        return
    for inst in hoist:
        main.instructions.remove(inst)
    for inst in reversed(hoist):
        idx = next(
            (
                i
                for i, pi in enumerate(pre.instructions)
                if isinstance(pi, mybir.InstDrain) and pi.engine == inst.engine
            ),
            None,
        )
        if idx is None:
            main.instructions.insert(0, inst)
        else:
            pre.instructions.insert(idx + 1, inst)


_NC_CACHE = None


def _get_nc():
    global _NC_CACHE
    if _NC_CACHE is None:
        _NC_CACHE = _build_nc()
    return _NC_CACHE


def _pack_inputs(Qs, Vs, cos32, sin32, idt):
    import ml_dtypes

    bf16 = ml_dtypes.bfloat16

    # [T, X] -> [P, NT, X] with t = p*NT + u, u = r*8 + c
    def r(x):
        return x.reshape(P, NT, -1)

    # compact tables: cos[p, r, c, k] = cos((p*16 + r*8 + c) * w_k)
    ce = r(cos32).reshape(P, 2, 8, HD)  # [p, r, c, k]
    se = r(sin32).reshape(P, 2, 8, HD)
    cosA = ce[:, 0].reshape(P, -1)
    sinA = se[:, 0].reshape(P, -1)
    cosB = ce[:, 1].reshape(P, -1)
    sinB = se[:, 1].reshape(P, -1)

    in_maps = []
    for core in range(N_CORES):
        h0 = core * HPC
        # q[p, r, x, h, c, k], v[p, c16, h, d]
        q = np.empty((P, 2, 2, HPC, 8, HD), np.float32)
        v = np.empty((P, NT, HPC, D), np.float32)
        for h in range(HPC):
            qh = r(Qs[h0 + h]).reshape(P, 2, 8, D)  # [p, r, c, d]
            q[:, :, 0, h] = qh[:, :, :, :HD]
            q[:, :, 1, h] = qh[:, :, :, HD:]
            v[:, :, h] = r(Vs[h0 + h])
        g1 = np.concatenate([cosA, sinA, q[:, 0, 0].reshape(P, -1)], axis=1)
        g2 = np.concatenate(
            [cosB, sinB, idt, q[:, 0, 1].reshape(P, -1)], axis=1
        )
        in_maps.append(
            {
                "G1": np.ascontiguousarray(g1.astype(bf16)),
                "G2": np.ascontiguousarray(g2.astype(bf16)),
                "G3": np.ascontiguousarray(
                    q[:, 1, 0].reshape(P, -1).astype(bf16)
                ),
                "G4": np.ascontiguousarray(
                    q[:, 1, 1].reshape(P, -1).astype(bf16)
                ),
                "VV": np.ascontiguousarray(v.reshape(P, -1).astype(bf16)),
            }
        )
    return in_maps


def _unpack_out(o):
    # o: [P, T] = outT; rows h*64+j, cols u-major: col = u*128 + f, t = f*16+u
    a = o.reshape(HPC, D, NT, P)  # [h, j, u, f]
    return a.transpose(0, 3, 2, 1).reshape(HPC, T, D)  # [h, t=f*16+u, j]


def run_inner(Q, K, V, trace=False):
    del K  # the module sets KR = QR; K is unused
    Qs = np.asarray(Q, dtype=np.float32)[0]  # [H, T, D]
    Vs = np.asarray(V, dtype=np.float32)[0]
    cos32, sin32 = _rope_tables()
    idt = np.eye(P, dtype=np.float32)
    nc = _get_nc()
    in_maps = _pack_inputs(Qs, Vs, cos32, sin32, idt)
    res = run_bass_kernel_spmd(nc, in_maps, list(range(N_CORES)), trace=trace)
    outs = [_unpack_out(np.asarray(res.results[i]["OUT"])) for i in range(N_CORES)]
    out = np.concatenate(outs, axis=0)[None]  # [1, H, T, D]
    return out.astype(np.float32), res


def kernel(Q, K, V):
    out, _ = run_inner(Q, K, V, trace=False)
    return out


# revision 28
# speedup vs baseline: 1.2464x; 1.0245x over previous
"""Trainium2 Bass kernel for nn_LinearAttention (RoPE(Q) @ RoPE(Q)^T @ V).

Algebra: no softmax, so out = (QR @ QR^T) @ V == QR @ (QR^T @ V) with a
[d,d] (64x64) intermediate per head. Sharding: 16 heads / 8 cores = 2
heads per core, no cross-core traffic; the two heads ride the two
64-lane halves of the 128x128 PE array. t = p*16 + r*8 + c (p = SBUF
partition, r = range, c = chunk); host packs/unpacks this permutation.

Key measured facts this version is built around (see trace notes):
  * Per DMA ring, each transfer costs ~1.8-2.8us end-to-end regardless
    of size (completion receipt does not pipeline), so inputs are
    grouped into at most 2 transfers per ring across 3 rings (sync,
    scalar-HWDGE, gpsimd-SWDGE), ordered by need:
      sync:   [cosA|sinA|qA-lo]  then [qB-lo]   (+ out blocks 0,2)
      scalar: [cosB|sinB|idt|qA-hi] then [qB-hi] (+ out blocks 1,3)
      gpsimd: [vA|vB]
  * Input DMA issues (~0.65us each on the issuing engine) are hoisted
    into the preamble block AFTER each engine's barrier-increment Drain
    (before it they stall every engine behind the issue train).
  * DVE tensor_tensor always needs the shared SBUF port that GpSimd
    locks per-instruction, so RoPE is DVE-only; cos/sin ship compact
    ([c,k], no per-head repeat) and broadcast over h via stride-0 APs.
  * matmul stationary APs allow one free dim -> qr is chunk-major,
    RoPE combines write strided.
  * Tile-pool recycling inserts completion semaphores: warm-up spam
    writes rotating slices of ONE preallocated PSUM bank instead.
  * PSUM evacuations batch 4 transpose chunks per bank and balance
    ACT/DVE; phase-3 uses 4 distinct PSUM banks.
  * Kernel tail: per-engine drains + one sem-only barrier, no sem
    clears (verified safe across re-executions).
"""

from contextlib import ExitStack

import numpy as np

import concourse.bass as bass
import concourse.mybir as mybir
import concourse.tile as tile
from concourse.bass_utils import run_bass_kernel_spmd
from concourse.vector_clock import ScopedClock

H, T, D = 16, 2048, 64
N_CORES = 8
HPC = H // N_CORES  # heads per core
P = 128
NT = T // P  # 16 t-chunks per core-head
HD = D // 2
F32 = mybir.dt.float32
BF16 = mybir.dt.bfloat16
N_WARM = 19  # leading dep-free matmuls: preamble -> first real MM
N_WARM_MID = 4  # bridge the PE gap while DVE finishes RoPE-B

# in_sb layout (elements per partition)
O_COSA, O_QALO = 0, 512
O_COSB, O_IDT, O_QAHI = 1024, 1536, 1664
O_QBLO, O_QBHI, O_V = 2176, 2688, 3200
N_IN = O_V + 2048


def _rope_tables():
    inv_freq = 1.0 / (10000.0 ** (np.arange(0, D, 2, dtype=np.float32) / D))
    t = np.arange(T, dtype=np.float32)
    freqs = np.outer(t, inv_freq).astype(np.float32)  # [T, D/2]
    return np.cos(freqs).astype(np.float32), np.sin(freqs).astype(np.float32)


class _SlimTileContext(tile.TileContext):
    """TileContext whose kernel tail uses per-engine drains + a single
    sem-only barrier instead of the full EVSEM butterfly (~8us) and the
    sem-clear chain (re-execution verified identical without it)."""

    def _drain_and_barrier(self, tick_clock, wait_clock):
        nc = self.nc
        drain_inst = nc.sync.drain()
        wait_clock.add_sem_waits(
            drain_inst.ins, ScopedClock({None: tick_clock.global_clock})
        )
        for eng in nc.engines.values():
            if eng.engine != mybir.EngineType.SP:
                eng.drain(fusable=False)
        nc.all_engine_barrier(sem_only=True)
        popped = nc._tile_sem_poison_stack.pop()
        assert popped is self._sem_poison


def _build_nc():
    nc = bass.Bass()
    G1 = nc.declare_dram_parameter("G1", [P, 1024], BF16, isOutput=False)
    G2 = nc.declare_dram_parameter("G2", [P, 1152], BF16, isOutput=False)
    G3 = nc.declare_dram_parameter("G3", [P, 512], BF16, isOutput=False)
    G4 = nc.declare_dram_parameter("G4", [P, 512], BF16, isOutput=False)
    VV = nc.declare_dram_parameter("VV", [P, 2048], BF16, isOutput=False)
    OUT = nc.declare_dram_parameter("OUT", [P, T], BF16, isOutput=True)

    with _SlimTileContext(nc) as tc, ExitStack() as ctx:
        singles = ctx.enter_context(tc.tile_pool(name="singles", bufs=1))
        ps_s = ctx.enter_context(tc.tile_pool(name="ps_s", bufs=1, space="PSUM"))
        ps_tp = ctx.enter_context(tc.tile_pool(name="ps_tp", bufs=2, space="PSUM"))
        ps_o = ctx.enter_context(tc.tile_pool(name="ps_o", bufs=4, space="PSUM"))
        ps_w = ctx.enter_context(tc.tile_pool(name="ps_w", bufs=1, space="PSUM"))

        in_sb = singles.tile([P, N_IN], BF16)
        # chunk-major so each chunk's (h,x,k) is a contiguous 128-elem
        # lhsT slice (matmul stationary APs allow only one free dim)
        qr_sb = singles.tile([P, NT, HPC, 2, HD], BF16)
        tm = singles.tile([P, 4, HPC, 8, HD], BF16)
        qrt_sb = singles.tile([P, NT * P], BF16)
        s2d = singles.tile([P, P], BF16)
        outT_sb = singles.tile([P, T], BF16)
        spam_src = singles.tile([P, P], F32)

        # V on the third (SWDGE) ring, at the head of the Pool queue --
        # it must NOT be hoisted before Pool's barrier-gather.
        nc.gpsimd.dma_start(
            out=in_sb[:, O_V : O_V + 2048], in_=VV[:]
        )

        # spam seed on DVE (idle until RoPE); s2d off-diagonal zeros on
        # GpSimd after the V issue (GpSimd must not overlap DVE
        # tensor_tensor work -- shared SBUF port).
        nc.vector.memset(spam_src[:, 0:2], 0.0)
        nc.gpsimd.memset(s2d[0:D, D:P], 0.0)
        nc.gpsimd.memset(s2d[D:P, 0:D], 0.0)

        # Two grouped transfers per HWDGE ring, hoisted into the
        # preamble block by _hoist_input_dmas.
        nc.sync.dma_start(out=in_sb[:, 0:1024], in_=G1[:])
        nc.scalar.dma_start(out=in_sb[:, 1024:2176], in_=G2[:])
        nc.sync.dma_start(out=in_sb[:, O_QBLO : O_QBLO + 512], in_=G3[:])
        nc.scalar.dma_start(out=in_sb[:, O_QBHI : O_QBHI + 512], in_=G4[:])

        # Garbage-input PE warm-up: dep-free REGULAR matmuls into
        # rotating slices of one preallocated PSUM bank (slices avoid
        # tile-pool recycling semaphores, which would serialize the PE
        # queue at ~531ns/op).
        spam_ps = ps_w.tile([P, 512], F32)
        for i in range(N_WARM):
            j = i % 4
            nc.tensor.matmul(
                spam_ps[:, j * P : (j + 1) * P], lhsT=spam_src, rhs=spam_src,
                start=True, stop=True, skip_group_check=True,
            )

        idt = in_sb[:, O_IDT : O_IDT + 128]
        v_view = in_sb[:, O_V : O_V + 2048].rearrange("p (c f) -> p c f", c=NT)

        def rope(r):
            # 6 contiguous [128,512] DVE ops; cos/sin broadcast over h.
            ocos = O_COSA if r == 0 else O_COSB
            cosB = (
                in_sb[:, ocos : ocos + 256]
                .rearrange("p (c k) -> p c k", c=8)
                .unsqueeze(1)
                .to_broadcast([P, HPC, 8, HD])
            )
            sinB = (
                in_sb[:, ocos + 256 : ocos + 512]
                .rearrange("p (c k) -> p c k", c=8)
                .unsqueeze(1)
                .to_broadcast([P, HPC, 8, HD])
            )
            olo = O_QALO if r == 0 else O_QBLO
            ohi = O_QAHI if r == 0 else O_QBHI
            qlo = in_sb[:, olo : olo + 512].rearrange(
                "p (h c k) -> p h c k", h=HPC, c=8
            )
            qhi = in_sb[:, ohi : ohi + 512].rearrange(
                "p (h c k) -> p h c k", h=HPC, c=8
            )
            cs = slice(r * 8, r * 8 + 8)
            # combine dests scatter into the chunk-major qr tile in the
            # same (h, c, k) iteration order as the contiguous srcs
            qr_lo = qr_sb[:, cs, :, 0, :].rearrange("p c h k -> p h c k")
            qr_hi = qr_sb[:, cs, :, 1, :].rearrange("p c h k -> p h c k")
            # q-lo muls first: the lo-half transfer lands first
            nc.vector.tensor_mul(tm[:, 0], qlo, cosB)
            nc.vector.tensor_mul(tm[:, 3], qlo, sinB)
            nc.vector.tensor_mul(tm[:, 1], qhi, sinB)
            nc.vector.tensor_sub(qr_lo, tm[:, 0], tm[:, 1])
            nc.vector.tensor_mul(tm[:, 2], qhi, cosB)
            nc.vector.tensor_add(qr_hi, tm[:, 2], tm[:, 3])

        s2_ps = ps_s.tile([P, P], F32)

        def phase2(r):
            # per chunk: one (shared) LDW + transpose MM + accum MM;
            # transposes batch 4 chunks per PSUM bank
            for ci in range(8):
                c = r * 8 + ci
                if c % 4 == 0:
                    phase2.tp = ps_tp.tile([P, 512], F32, tag="tp")
                tp = phase2.tp
                qr_c = qr_sb[:, c].rearrange("p h x k -> p (h x k)")
                j = c % 4
                # transpose first: a late V then doesn't stall the
                # transpose/evac pipeline behind it in the PE FIFO
                nc.tensor.matmul(
                    tp[:, j * P : (j + 1) * P], lhsT=qr_c, rhs=idt,
                    start=True, stop=True,
                )
                nc.tensor.matmul(
                    s2_ps, lhsT=qr_c, rhs=v_view[:, c],
                    start=(c == 0), stop=(c == NT - 1),
                )
                if c % 4 == 3:
                    # ACT takes groups 0-2 (DVE busy with RoPE); DVE
                    # (free after RoPE-B) takes the last group
                    g = c // 4
                    dst = qrt_sb[:, g * 512 : (g + 1) * 512]
                    if g < 3:
                        nc.scalar.copy(out=dst, in_=tp)
                    else:
                        # s2d diag cast FIRST in the DVE queue: it
                        # gates phase 3 and must not sit behind the
                        # 0.7us g3 evacuation
                        nc.vector.tensor_copy(
                            out=s2d[D:P, D:P], in_=s2_ps[D:P, D:P]
                        )
                        nc.vector.tensor_copy(out=dst, in_=tp)

        rope(0)
        phase2(0)
        rope(1)
        # Bridge the PE idle window while DVE finishes RoPE-B.
        for i in range(N_WARM_MID):
            j = i % 4
            nc.tensor.matmul(
                spam_ps[:, j * P : (j + 1) * P], lhsT=spam_src, rhs=spam_src,
                start=True, stop=True, skip_group_check=True,
            )
        phase2(1)

        # Other diagonal S_h block on ACT (the DVE half is emitted
        # inside phase2(1) ahead of the g3 evacuation).
        nc.scalar.copy(out=s2d[0:D, 0:D], in_=s2_ps[0:D, 0:D])

        # outT blocks: blockdiag(S)^T @ QRT serves both heads at once.
        # 4 distinct PSUM banks; evac casts alternate DVE/ACT; output
        # DMAs alternate the two HWDGE rings.
        for i in range(4):
            o_ps = ps_o.tile([P, 512], F32, tag="o")
            blk = slice(i * 512, (i + 1) * 512)
            nc.tensor.matmul(
                o_ps, lhsT=s2d, rhs=qrt_sb[:, blk], start=True, stop=True
            )
            if i % 2 == 0:
                nc.vector.tensor_copy(out=outT_sb[:, blk], in_=o_ps)
                nc.sync.dma_start(out=OUT[:, blk], in_=outT_sb[:, blk])
            else:
                nc.scalar.copy(out=outT_sb[:, blk], in_=o_ps)
                nc.scalar.dma_start(out=OUT[:, blk], in_=outT_sb[:, blk])

    _split_multi_waits(nc)
    _hoist_input_dmas(nc)
    return nc


def _split_multi_waits(nc):
    """This compiler build rejects instructions carrying more than one
    sync-wait command: split extras into single-wait NoOps placed
    immediately before on the same engine."""
    n = 0
    for f in nc.m.functions:
        for blk in f.blocks:
            new_insts = []
            for inst in blk.instructions:
                si = inst.sync_info
                waits = list(si.on_wait) if si else []
                if len(waits) > 1:
                    for w in waits[:-1]:
                        nop = mybir.InstNoOp(name=f"W-split-{n}", ins=[], outs=[])
                        n += 1
                        nop.engine = inst.engine
                        nop.sync_info = mybir.SyncInfo(on_wait=[w], on_update=[])
                        new_insts.append(nop)
                    inst.sync_info = mybir.SyncInfo(
                        on_wait=[waits[-1]], on_update=list(si.on_update)
                    )
                new_insts.append(inst)
            blk.instructions = new_insts


def _hoist_input_dmas(nc):
    """Move the wait-free input DMA issues from the main block into the
    preamble block, right AFTER each issuing engine's barrier-increment
    Drain (a DMA issue costs ~0.65us on the issuing engine; placing it
    before the Drain would hold the all-engine barrier and stall every
    other engine behind the whole DMA issue train)."""
    f = nc.m.functions[0]
    if len(f.blocks) < 2:
        return
    pre, main = f.blocks[0], f.blocks[1]
    hoist = []
    for inst in list(main.instructions):
        if isinstance(inst, mybir.InstDMACopy):
            if inst.engine == mybir.EngineType.Pool:
                # Pool is the barrier gatherer: a DMA issue before its
                # gather EventSemaphores would stall every engine
                continue
            si = inst.sync_info
            if si is not None and len(si.on_wait) > 0:
                continue
            srcs = [x.memref for x in inst.ins] if inst.ins else []
            if any(n.startswith(("G1", "G2", "G3", "G4")) for n in srcs):
                hoist.append(inst)
    if not hoist:
        return
    for inst in hoist:
        main.instructions.remove(inst)
    for inst in reversed(hoist):
        idx = next(
            (
                i
                for i, pi in enumerate(pre.instructions)
                if isinstance(pi, mybir.InstDrain) and pi.engine == inst.engine
            ),
            None,
        )
        if idx is None:
            main.instructions.insert(0, inst)
        else:
            pre.instructions.insert(idx + 1, inst)


_NC_CACHE = None


def _get_nc():
    global _NC_CACHE
    if _NC_CACHE is None:
        _NC_CACHE = _build_nc()
    return _NC_CACHE


def _pack_inputs(Qs, Vs, cos32, sin32, idt):
    import ml_dtypes

    bf16 = ml_dtypes.bfloat16

    # [T, X] -> [P, NT, X] with t = p*NT + u, u = r*8 + c
    def r(x):
        return x.reshape(P, NT, -1)

    # compact tables: cos[p, r, c, k] = cos((p*16 + r*8 + c) * w_k)
    ce = r(cos32).reshape(P, 2, 8, HD)  # [p, r, c, k]
    se = r(sin32).reshape(P, 2, 8, HD)
    cosA = ce[:, 0].reshape(P, -1)
    sinA = se[:, 0].reshape(P, -1)
    cosB = ce[:, 1].reshape(P, -1)
    sinB = se[:, 1].reshape(P, -1)

    in_maps = []
    for core in range(N_CORES):
        h0 = core * HPC
        # q[p, r, x, h, c, k], v[p, c16, h, d]
        q = np.empty((P, 2, 2, HPC, 8, HD), np.float32)
        v = np.empty((P, NT, HPC, D), np.float32)
        for h in range(HPC):
            qh = r(Qs[h0 + h]).reshape(P, 2, 8, D)  # [p, r, c, d]
            q[:, :, 0, h] = qh[:, :, :, :HD]
            q[:, :, 1, h] = qh[:, :, :, HD:]
            v[:, :, h] = r(Vs[h0 + h])
        g1 = np.concatenate([cosA, sinA, q[:, 0, 0].reshape(P, -1)], axis=1)
        g2 = np.concatenate(
            [cosB, sinB, idt, q[:, 0, 1].reshape(P, -1)], axis=1
        )
        in_maps.append(
            {
                "G1": np.ascontiguousarray(g1.astype(bf16)),
                "G2": np.ascontiguousarray(g2.astype(bf16)),
                "G3": np.ascontiguousarray(
                    q[:, 1, 0].reshape(P, -1).astype(bf16)
                ),
                "G4": np.ascontiguousarray(
                    q[:, 1, 1].reshape(P, -1).astype(bf16)
                ),
                "VV": np.ascontiguousarray(v.reshape(P, -1).astype(bf16)),
            }
        )
    return in_maps


def _unpack_out(o):
    # o: [P, T] = outT; rows h*64+j, cols u-major: col = u*128 + f, t = f*16+u
    a = o.reshape(HPC, D, NT, P)  # [h, j, u, f]
    return a.transpose(0, 3, 2, 1).reshape(HPC, T, D)  # [h, t=f*16+u, j]


def run_inner(Q, K, V, trace=False):
    del K  # the module sets KR = QR; K is unused
    Qs = np.asarray(Q, dtype=np.float32)[0]  # [H, T, D]
    Vs = np.asarray(V, dtype=np.float32)[0]
    cos32, sin32 = _rope_tables()
    idt = np.eye(P, dtype=np.float32)
    nc = _get_nc()
    in_maps = _pack_inputs(Qs, Vs, cos32, sin32, idt)
    res = run_bass_kernel_spmd(nc, in_maps, list(range(N_CORES)), trace=trace)
    outs = [_unpack_out(np.asarray(res.results[i]["OUT"])) for i in range(N_CORES)]
    out = np.concatenate(outs, axis=0)[None]  # [1, H, T, D]
    return out.astype(np.float32), res


def kernel(Q, K, V):
    out, _ = run_inner(Q, K, V, trace=False)
    return out
